# revision 1
# baseline (speedup 1.0000x reference)
"""ColAttention TRN2 kernel: 8-core data-parallel over batch (2 batches/core).

Math (per batch b, width-column w):
  Q = Wq@x+bq; K = Wk@x+bk; V = Wv@x+bv        (1x1 convs over c)
  S[h,g] = sum_q Q[q,h]K[q,g]; attn = softmax_g(S)
  out = gamma * (V @ attn^T) + x

Device pipeline (all matmuls bf16, fp32 PSUM accumulation):
  host folds bv/gamma*bv via e = gamma*(I+gamma*Wv)^-1 bv:  xb = x+e,
  bq' = bq-Wq@e, bk' = bk-Wk@e  =>  device never touches bv and the
  residual add of xb reproduces the reference exactly (algebra in notes).
  A : Q,K projections (batched over h*w, N=384 chunks)
  B1: per column: S^T[g,h] via MM(lhsT=K_col, rhs=Q_col); exp via ACT
      (no max-subtraction: |S|<~50 so exp stays finite in f32);
      colsum^T[h] via MM(lhsT=expS_col, rhs=ones); 1/colsum on DVE;
      V^T[g,c] via MM(lhsT=x_col, rhs=Wv^T)
  B2: transpose recip^T[h,w-half] -> recipW[w,h] on PE
  B3: bcast tile gamma*r[h] over 128 partitions via K=1 outer-product MM;
      U[c,h] = MM(lhsT=V^T, rhs=expS); final = U*bcast + xb; DMA out.
"""
import sys

sys.path.insert(0, "/opt/trn_rl_repo")

import numpy as np
import ml_dtypes

import concourse.bass as bass
import concourse.bacc as bacc
import concourse.mybir as mybir
import concourse.tile as tile
from concourse.bass_utils import run_bass_kernel_spmd

F32 = mybir.dt.float32
BF16 = mybir.dt.bfloat16
AF = mybir.ActivationFunctionType

P = 128
H = 96          # height = attention sequence length
W = 96          # width  = independent columns
HW = H * W
B_LOC = 2       # batches per core
WH = 48         # columns per w-half
WC = 4          # columns per B-chunk
NCH = WH // WC  # 12 chunks per w-half

# engines for the Q/K projection evacuation, round-robined
_QK_EVAC = ("vector",)
# engine for the final residual add
TTADD_ENGINE = "vector"


def _build():
    nc = bacc.Bacc("TRN2", target_bir_lowering=False, debug=False)

    xb_d = nc.dram_tensor("xb", [B_LOC, 2, P, HW], BF16, kind="ExternalInput")
    cb_d = nc.dram_tensor("cblob", [P, 866], BF16, kind="ExternalInput")
    bb_d = nc.dram_tensor("bblob", [P, 2], F32, kind="ExternalInput")
    out_d = nc.dram_tensor("out", [B_LOC, 2, P, HW], F32, kind="ExternalOutput")

    with tile.TileContext(nc) as tc:
        import contextlib

        ctx = contextlib.ExitStack()
        with ctx:
            consts = ctx.enter_context(tc.tile_pool(name="consts", bufs=1))
            xp = ctx.enter_context(tc.tile_pool(name="xp", bufs=1))
            qkp = ctx.enter_context(tc.tile_pool(name="qkp", bufs=2))
            esp = ctx.enter_context(tc.tile_pool(name="esp", bufs=2))
            vtp = ctx.enter_context(tc.tile_pool(name="vtp", bufs=1))
            rtp = ctx.enter_context(tc.tile_pool(name="rtp", bufs=2))
            fp = ctx.enter_context(tc.tile_pool(name="fp", bufs=1))
            bctp = ctx.enter_context(tc.tile_pool(name="bctp", bufs=3))
            ttp = ctx.enter_context(tc.tile_pool(name="ttp", bufs=3))
            ps = ctx.enter_context(tc.tile_pool(name="ps", bufs=2, space="PSUM"))

            cb_t = consts.tile([P, 866], BF16)
            bb_t = consts.tile([P, 2], F32)
            nc.sync.dma_start(out=cb_t, in_=cb_d.ap())
            nc.sync.dma_start(out=bb_t, in_=bb_d.ap())
            # observers: funnel DMA deps into one engine each (this walrus
            # accepts a single semaphore wait per instruction)
            nc.tensor.ldweights(cb_t[:, 0:128])
            bias_t = consts.tile([P, 2], F32)
            nc.vector.tensor_copy(bias_t, bb_t)
            wq_t = cb_t[:, 0:128].rearrange("p (c m) -> p c m", c=2)
            wk_t = cb_t[:, 128:256].rearrange("p (c m) -> p c m", c=2)
            wvt_t = cb_t[:, 256:768].rearrange("p (c m) -> p c m", c=2)
            bq_t = bias_t[0:64, 0:1]
            bk_t = bias_t[64:128, 0:1]
            gvec_t = bias_t[0:H, 1:2]
            invg_t = cb_t[0:H, 769:770]
            idb_t = cb_t[0:H, 770:866]

            for b in range(B_LOC):
                x_cm = xp.tile([P, 2, HW], BF16, tag="xcm")
                for ci in range(2):
                    nc.sync.dma_start(out=x_cm[:, ci, :], in_=xb_d.ap()[b, ci])
                # h-major views (x_cm holds h-major data in this variant)
                x_cols = [
                    x_cm[:, ci, :].rearrange("p (h w) -> p w h", w=W) for ci in range(2)
                ]
                x_rows = [
                    x_cm[:, ci, :].rearrange("p (h w) -> p h w", w=W) for ci in range(2)
                ]
                f_ts = [fp.tile([P, HW], F32, tag=f"f{ci}", name=f"f{ci}") for ci in range(2)]
                for f in f_ts:
                    nc.vector.memset(f[0:1, 0:1], 0.0)
                f_cols = [f.rearrange("p (h w) -> p w h", w=W) for f in f_ts]

                for half in range(2):
                    # ---- A: Q/K projections for this w-half -------------------
                    q_t = qkp.tile([64, WH * H], BF16, tag="q", bufs=1)
                    k_t = qkp.tile([64, WH * H], BF16, tag="k", bufs=1)
                    ei = 0
                    for (w_l, b_l, o_t) in ((wq_t, bq_t, q_t), (wk_t, bk_t, k_t)):
                        for hc in range(12):  # 8 h-rows x 48 cols = N=384
                            pr = ps.tile([64, 384], F32, tag="s", bufs=1)
                            for ci in range(2):
                                rhs = x_rows[ci][
                                    :, hc * 8 : (hc + 1) * 8, half * WH : (half + 1) * WH
                                ]
                                nc.tensor.matmul(
                                    pr, w_l[:, ci, :], rhs,
                                    start=(ci == 0), stop=(ci == 1),
                                )
                            dst = o_t[:, hc * 384 : (hc + 1) * 384]
                            if _QK_EVAC[ei % len(_QK_EVAC)] == "act":
                                nc.scalar.activation(
                                    out=dst, in_=pr, func=AF.Identity, bias=b_l, scale=1.0
                                )
                            else:
                                nc.vector.tensor_scalar(
                                    out=dst, in0=pr, scalar1=b_l, scalar2=None,
                                    op0=mybir.AluOpType.add,
                                )
                            ei += 1
                    q_cols = q_t.rearrange("p (h w) -> p w h", w=WH)
                    k_cols = k_t.rearrange("p (h w) -> p w h", w=WH)

                    # ---- B1: scores/exp/colsum/recip + V^T --------------------
                    es_t = esp.tile([H, WH * H], BF16, tag="es", bufs=1)
                    vt_t = vtp.tile([H, WH, 256], BF16, tag="vt")
                    rt_t = rtp.tile([H, WH], F32, tag="rt")
                    for ch in range(NCH):
                        s_t = ps.tile([H, WC * H], F32, tag="s", bufs=1)
                        for j in range(WC):
                            wl = ch * WC + j
                            nc.tensor.matmul(
                                s_t[:, j * H : (j + 1) * H],
                                k_cols[:, wl, :], q_cols[:, wl, :],
                                start=True, stop=True,
                            )
                        es_ch = es_t[:, ch * WC * H : (ch + 1) * WC * H]
                        nc.scalar.activation(out=es_ch, in_=s_t[:, :], func=AF.Exp)
                        cs_p = ps.tile([H, WC], F32, tag="cs", bufs=1)
                        for j in range(WC):
                            wl = ch * WC + j
                            nc.tensor.matmul(
                                cs_p[:, j : j + 1],
                                es_t[:, wl * H : (wl + 1) * H], invg_t,
                                start=True, stop=True,
                            )
                        nc.vector.reciprocal(
                            out=rt_t[:, ch * WC : (ch + 1) * WC], in_=cs_p
                        )
                        for pair in range(2):
                            vp = ps.tile([H, 512], F32, tag="vtp", bufs=1)
                            for j2 in range(2):
                                wl = ch * WC + pair * 2 + j2
                                for ci in range(2):
                                    nc.tensor.matmul(
                                        vp[:, j2 * 256 : (j2 + 1) * 256],
                                        x_cols[ci][:, half * WH + wl, :],
                                        wvt_t[:, ci, :],
                                        start=(ci == 0), stop=(ci == 1),
                                    )
                            nc.scalar.copy(
                                out=vt_t[:, ch * WC + pair * 2 : ch * WC + pair * 2 + 2, :],
                                in_=vp,
                            )

                    # ---- B2: gamma/colsum as bf16 for the bcast matmul --------
                    rtb_t = rtp.tile([H, WH], BF16, tag="rw")
                    nc.vector.tensor_copy(rtb_t, rt_t)

                    # ---- B3: bcast, U, normalize, residual --------------------
                    for ch in range(NCH):
                        bcp = ps.tile([P, WC * H], F32, tag="bcp", bufs=2)
                        for j in range(WC):
                            wl = ch * WC + j
                            nc.tensor.matmul(
                                bcp[:, j * H : (j + 1) * H],
                                rtb_t[:, wl : wl + 1].to_broadcast([H, P]),
                                idb_t, start=True, stop=True,
                            )
                        bc_t = bctp.tile([P, WC * H], BF16, tag="bc")
                        nc.vector.tensor_copy(bc_t, bcp)
                        bc3 = bc_t.rearrange("p (c h) -> p c h", h=H)
                        for ci in range(2):
                            u_t = ps.tile([P, WC * H], F32, tag="u", bufs=3)
                            u3 = u_t.rearrange("p (c k) -> p c k", k=H)
                            for j in range(WC):
                                wl = ch * WC + j
                                nc.tensor.matmul(
                                    u_t[:, j * H : (j + 1) * H],
                                    vt_t[:, wl, ci * 128 : (ci + 1) * 128],
                                    es_t[:, wl * H : (wl + 1) * H],
                                    start=True, stop=True,
                                )
                            t_t = ttp.tile([P, WC * H], BF16, tag="t")
                            t3 = t_t.rearrange("p (c h) -> p c h", h=H)
                            nc.vector.tensor_mul(t3, u3, bc3)
                            xslice = x_cols[ci][
                                :, half * WH + ch * WC : half * WH + (ch + 1) * WC, :
                            ]
                            fslice = f_cols[ci][
                                :, half * WH + ch * WC : half * WH + (ch + 1) * WC, :
                            ]
                            eng = getattr(nc, TTADD_ENGINE)
                            eng.tensor_add(fslice, t3, xslice)

                for ci in range(2):
                    nc.sync.dma_start(out=out_d.ap()[b, ci], in_=f_ts[ci])
    nc.compile()
    return nc


_NC_CACHE = None


def _get_nc():
    global _NC_CACHE
    if _NC_CACHE is None:
        _NC_CACHE = _build()
    return _NC_CACHE


def kernel(x, Wq, bq, Wk, bk, Wv, bv, gamma):
    x = np.asarray(x, np.float32)
    Wq = np.asarray(Wq, np.float32)
    bq = np.asarray(bq, np.float32)
    Wk = np.asarray(Wk, np.float32)
    bk = np.asarray(bk, np.float32)
    Wv = np.asarray(Wv, np.float32)
    bv = np.asarray(bv, np.float32)
    g = float(np.asarray(gamma, np.float32)[0])

    C = 256
    e = (g * np.linalg.solve(np.eye(C, dtype=np.float64) + g * Wv.astype(np.float64),
                             bv.astype(np.float64))).astype(np.float32)
    xb = (x + e[None, :, None, None]).astype(ml_dtypes.bfloat16)
    xb = np.ascontiguousarray(xb).reshape(16, 2, P, HW)

    cblob = np.zeros((P, 866), np.float32)
    cblob[:, 0:128] = np.stack([Wq[:, :128].T, Wq[:, 128:].T], axis=1).reshape(P, 128)
    cblob[:, 128:256] = np.stack([Wk[:, :128].T, Wk[:, 128:].T], axis=1).reshape(P, 128)
    cblob[:, 256:768] = np.stack([Wv[:, :128].T, Wv[:, 128:].T], axis=1).reshape(P, 512)
    cblob[0:H, 769] = 1.0 / g
    cblob[0:H, 770:866] = np.eye(H, dtype=np.float32)
    cblob = cblob.astype(ml_dtypes.bfloat16)
    bblob = np.zeros((P, 2), np.float32)
    bblob[0:64, 0] = bq - Wq @ e
    bblob[64:128, 0] = bk - Wk @ e
    bblob[0:H, 1] = g

    nc = _get_nc()
    in_maps = []
    for core in range(8):
        in_maps.append({
            "xb": xb[core * B_LOC : (core + 1) * B_LOC],
            "cblob": cblob, "bblob": bblob,
        })
    res = run_bass_kernel_spmd(nc, in_maps, core_ids=list(range(8)))
    outs = [r["out"].reshape(B_LOC, C, H, W) for r in res.results]
    return np.concatenate(outs, axis=0)


def prepared_in_maps(inputs):
    """test-harness helper: the per-core in_maps for a full input dict."""
    import inspect
    sig = ("x", "Wq", "bq", "Wk", "bk", "Wv", "bv", "gamma")
    global _CAPTURE
    _CAPTURE = None
    # rebuild the same host prep by calling kernel body up to run: duplicate code
    x = np.asarray(inputs["x"], np.float32)
    Wq = np.asarray(inputs["Wq"], np.float32); bq = np.asarray(inputs["bq"], np.float32)
    Wk = np.asarray(inputs["Wk"], np.float32); bk = np.asarray(inputs["bk"], np.float32)
    Wv = np.asarray(inputs["Wv"], np.float32); bv = np.asarray(inputs["bv"], np.float32)
    g = float(np.asarray(inputs["gamma"], np.float32)[0])
    C = 256
    e = (g * np.linalg.solve(np.eye(C, dtype=np.float64) + g * Wv.astype(np.float64),
                             bv.astype(np.float64))).astype(np.float32)
    xb = (x + e[None, :, None, None]).astype(ml_dtypes.bfloat16)
    xb = np.ascontiguousarray(xb).reshape(16, 2, P, HW)
    cblob = np.zeros((P, 866), np.float32)
    cblob[:, 0:128] = np.stack([Wq[:, :128].T, Wq[:, 128:].T], axis=1).reshape(P, 128)
    cblob[:, 128:256] = np.stack([Wk[:, :128].T, Wk[:, 128:].T], axis=1).reshape(P, 128)
    cblob[:, 256:768] = np.stack([Wv[:, :128].T, Wv[:, 128:].T], axis=1).reshape(P, 512)
    cblob[0:H, 769] = 1.0 / g
    cblob[0:H, 770:866] = np.eye(H, dtype=np.float32)
    cblob = cblob.astype(ml_dtypes.bfloat16)
    bblob = np.zeros((P, 2), np.float32)
    bblob[0:64, 0] = bq - Wq @ e
    bblob[64:128, 0] = bk - Wk @ e
    bblob[0:H, 1] = g
    return [
        {"xb": xb[c * B_LOC : (c + 1) * B_LOC], "cblob": cblob, "bblob": bblob}
        for c in range(8)
    ]



# revision 2
# speedup vs baseline: 1.2103x; 1.2103x over previous
"""ColAttention TRN2 kernel v2: 8-core data-parallel over batch (2 batches/core).

Math (per batch b, width-column w):
  Q = Wq@x+bq; K = Wk@x+bk; V = Wv@x+bv        (1x1 convs over c)
  S[h,g] = sum_q Q[q,h]K[q,g]; attn = softmax_g(S)
  out = gamma * (V @ attn^T) + x

v2 design (all layouts w-major: free dim = w*H + h, columns contiguous):
  host folds bv via e = gamma*(I+gamma*Wv)^-1 bv: xb = x+e, bq' = bq-Wq@e,
  bk' = bk-Wk@e; host also prepares xT (per-column transposed x) so the
  V-path runs as out = Wv @ (xb_col @ attn^T) (associativity swap):
  A : Q,K projections, M=64 N=512 chunks, ACT-evac with bias
  B : per 4-column chunk:
      4x scores mm S^T[g,h] (lhsT=k_col, rhs=q_col, contiguous)
      exp on ACT -> es bf16
      colsum as ONE mm: lhsT=ones*(1/gamma) [96,1] const, rhs=es chunk -> cs[1,384]
      recip on DVE -> r[1,384] bf16
      bcast as ONE K=1 mm: lhsT=ones[1,96], rhs=r -> bc[96,384] psum
      es_n = es * bc on DVE (folds softmax normalization + gamma into es)
      8x Y mm: y[c',h] = sum_g xT_col[g,c'] es_n[g,h] (lhsT=xT col-half)
      2x y evac psum->sbuf bf16
      4x U mm: u[c,h] = Wv^T-quarters @ y (const weights, M=128 K=128 N=384)
      residual: f = u + xb chunk (DVE for c-half 0, GPSIMD for c-half 1)
      DMA f -> out (f32)
"""
import sys

sys.path.insert(0, "/opt/trn_rl_repo")

import numpy as np
import ml_dtypes

import concourse.bass as bass
import concourse.bacc as bacc
import concourse.mybir as mybir
import concourse.tile as tile
from concourse.bass_utils import run_bass_kernel_spmd

F32 = mybir.dt.float32
BF16 = mybir.dt.bfloat16
AF = mybir.ActivationFunctionType
OP = mybir.AluOpType

P = 128
H = 96          # height = attention sequence length
W = 96          # width  = independent columns
HW = H * W
B_LOC = 2       # batches per core
WH = 48         # columns per w-half
HHW = WH * H    # free elems per half = 4608
WC = 4          # columns per chunk
NCH = WH // WC  # 12 chunks per half
CHN = WC * H    # chunk free width = 384

# ---- knobs ----
ADD_ENGINES = ("vector", "vector")   # residual add engine per c-half (mi)
                                     # (gpsimd cannot read PSUM)
YEVAC_ENGINES = ("scalar", "vector")  # y psum->sbuf evac engine per ci
QK_EVAC = "scalar"                   # q/k psum->sbuf + bias engine


def _build():
    nc = bacc.Bacc("TRN2", target_bir_lowering=False, debug=False)

    xw_d = nc.dram_tensor("xw", [B_LOC, 2, P, HW], BF16, kind="ExternalInput")
    xt_d = nc.dram_tensor("xt", [B_LOC, H, W * 256], BF16, kind="ExternalInput")
    cb_d = nc.dram_tensor("cblob", [P, 866], BF16, kind="ExternalInput")
    bb_d = nc.dram_tensor("bblob", [64, 2], F32, kind="ExternalInput")
    out_d = nc.dram_tensor("out", [B_LOC, 2, P, HW], BF16, kind="ExternalOutput")

    with tile.TileContext(nc) as tc:
        import contextlib

        ctx = contextlib.ExitStack()
        with ctx:
            consts = ctx.enter_context(tc.tile_pool(name="consts", bufs=1))
            xp = ctx.enter_context(tc.tile_pool(name="xp", bufs=2))
            xtp = ctx.enter_context(tc.tile_pool(name="xtp", bufs=2))
            qkp = ctx.enter_context(tc.tile_pool(name="qkp", bufs=2))
            esp = ctx.enter_context(tc.tile_pool(name="esp", bufs=3))
            rp = ctx.enter_context(tc.tile_pool(name="rp", bufs=3))
            ysp = ctx.enter_context(tc.tile_pool(name="ysp", bufs=4))
            fp = ctx.enter_context(tc.tile_pool(name="fp", bufs=4))
            ps = ctx.enter_context(tc.tile_pool(name="ps", bufs=2, space="PSUM"))

            cb_t = consts.tile([P, 866], BF16)
            bb_t = consts.tile([64, 2], F32)
            nc.sync.dma_start(out=cb_t, in_=cb_d.ap())
            nc.sync.dma_start(out=bb_t, in_=bb_d.ap())
            # observers: funnel DMA deps into one engine each
            import os
            if os.environ.get("K2_NO_LDW_OBSERVER") != "1":
                nc.tensor.ldweights(cb_t[:, 0:128])
            bias_t = consts.tile([64, 2], F32)
            nc.scalar.copy(bias_t, bb_t)

            wq_t = cb_t[:, 0:128].rearrange("p (c m) -> p c m", c=2)    # [128,2,64]
            wk_t = cb_t[:, 128:256].rearrange("p (c m) -> p c m", c=2)  # [128,2,64]
            # wv quarters: offset 256 + (ci*2+mi)*128
            wv_t = cb_t[:, 256:768].rearrange("p (c m) -> p c m", c=4)  # [128,4,128]
            ones_gg = cb_t[0:H, 768:864]     # [96,96] = 1/gamma (sum+bcast lhsT)
            bq_t = bias_t[:, 0:1]
            bk_t = bias_t[:, 1:2]

            for b in range(B_LOC):
                for half in range(2):
                    x_t = xp.tile([P, 2, HHW], BF16, tag="x")
                    for ci in range(2):
                        for piece in range(3):
                            nc.sync.dma_start(
                                out=x_t[:, ci, piece * 1536:(piece + 1) * 1536],
                                in_=xw_d.ap()[b, ci][
                                    :, half * HHW + piece * 1536:
                                    half * HHW + (piece + 1) * 1536],
                            )
                    xt_t = xtp.tile([H, WH * 256], BF16, tag="xt")
                    for piece in range(2):
                        nc.sync.dma_start(
                            out=xt_t[:, piece * 6144:(piece + 1) * 6144],
                            in_=xt_d.ap()[b][
                                :, half * WH * 256 + piece * 6144:
                                half * WH * 256 + (piece + 1) * 6144],
                        )
                    xt3 = xt_t.rearrange("p (w c) -> p w c", c=256)

                    # ---- A: Q/K projections (M=64, N=512) -------------------
                    q_t = qkp.tile([64, HHW], BF16, tag="q")
                    k_t = qkp.tile([64, HHW], BF16, tag="k")
                    for (w_l, b_l, o_t) in ((wq_t, bq_t, q_t), (wk_t, bk_t, k_t)):
                        for n in range(9):
                            pr = ps.tile([64, 512], F32, tag="s", bufs=3)
                            for ci in range(2):
                                nc.tensor.matmul(
                                    pr, w_l[:, ci, :],
                                    x_t[:, ci, n * 512:(n + 1) * 512],
                                    start=(ci == 0), stop=(ci == 1),
                                )
                            nc.scalar.activation(
                                out=o_t[:, n * 512:(n + 1) * 512], in_=pr,
                                func=AF.Identity, bias=b_l, scale=1.0,
                            )

                    # ---- B: software-pipelined per-chunk attention ----------
                    # stage A(k): scores + exp; B(k-1): sum/bcast mm + recip
                    # + normalize; C(k-2): Y, evac, U, residual, DMA out.
                    es_ts = {}
                    esn_ts = {}
                    y_tss = {}
                    f_ts = [fp.tile([P, HHW], BF16, tag="fh", name=f"fh{i}")
                            for i in range(2)]
                    for k in range(NCH + 3):
                        if k < NCH:
                            s_t = ps.tile([H, CHN], F32, tag="s", bufs=3)
                            for j in range(WC):
                                wl = k * WC + j
                                nc.tensor.matmul(
                                    s_t[:, j * H:(j + 1) * H],
                                    k_t[:, wl * H:(wl + 1) * H],
                                    q_t[:, wl * H:(wl + 1) * H],
                                    start=True, stop=True,
                                )
                            es_t = esp.tile([H, CHN], BF16, tag="es")
                            nc.scalar.activation(out=es_t, in_=s_t, func=AF.Exp)
                            es_ts[k] = es_t
                        if 1 <= k <= NCH:
                            ch = k - 1
                            # colsum over g, pre-broadcast over 96 partitions,
                            # scaled by 1/gamma: one mm with const lhsT
                            csb_p = ps.tile([H, CHN], F32, tag="csb", bufs=1)
                            nc.tensor.matmul(
                                csb_p, ones_gg, es_ts[ch], start=True, stop=True)
                            rb_t = rp.tile([H, CHN], F32, tag="rb")
                            nc.vector.reciprocal_approx_fast(out=rb_t, in_=csb_p)
                            esn_t = esp.tile([H, CHN], BF16, tag="esn")
                            nc.gpsimd.tensor_mul(esn_t, es_ts.pop(ch), rb_t)
                            esn_ts[ch] = esn_t
                        if 2 <= k <= NCH + 1:
                            ch = k - 2
                            esn_t = esn_ts.pop(ch)
                            y_ps = [
                                ps.tile([P, CHN], F32, tag="y", bufs=2,
                                        name=f"yp{i}")
                                for i in range(2)
                            ]
                            for j in range(WC):
                                wl = ch * WC + j
                                for mi in range(2):
                                    nc.tensor.matmul(
                                        y_ps[mi][:, j * H:(j + 1) * H],
                                        xt3[:, wl, mi * 128:(mi + 1) * 128],
                                        esn_t[:, j * H:(j + 1) * H],
                                        start=True, stop=True,
                                    )
                            y_ts = []
                            for ci in range(2):
                                y_t = ysp.tile([P, CHN], BF16, tag="ysb")
                                if YEVAC_ENGINES[ci] == "scalar":
                                    nc.scalar.copy(out=y_t, in_=y_ps[ci])
                                else:
                                    getattr(nc, YEVAC_ENGINES[ci]).tensor_copy(
                                        y_t, y_ps[ci])
                                y_ts.append(y_t)
                            y_tss[ch] = y_ts

                        if 3 <= k:
                            ch = k - 3
                            y_ts = y_tss.pop(ch)
                            # U: Wv^T quarters (const weights) @ y
                            for mi in range(2):
                                u_p = ps.tile([P, CHN], F32, tag="u", bufs=2)
                                for ci in range(2):
                                    nc.tensor.matmul(
                                        u_p, wv_t[:, ci * 2 + mi, :], y_ts[ci],
                                        start=(ci == 0), stop=(ci == 1),
                                    )
                                eng = getattr(nc, ADD_ENGINES[mi])
                                eng.tensor_add(
                                    f_ts[mi][:, ch * CHN:(ch + 1) * CHN], u_p,
                                    x_t[:, mi, ch * CHN:(ch + 1) * CHN],
                                )
                    for mi in range(2):
                        for piece in range(2):
                            nc.sync.dma_start(
                                out=out_d.ap()[b, mi][
                                    :, half * HHW + piece * 2304:
                                    half * HHW + (piece + 1) * 2304],
                                in_=f_ts[mi][:, piece * 2304:(piece + 1) * 2304],
                            )
    nc.compile()
    return nc


_NC_CACHE = None


def _get_nc():
    global _NC_CACHE
    if _NC_CACHE is None:
        _NC_CACHE = _build()
    return _NC_CACHE


def _prep(x, Wq, bq, Wk, bk, Wv, bv, gamma):
    x = np.asarray(x, np.float32)
    Wq = np.asarray(Wq, np.float32)
    bq = np.asarray(bq, np.float32)
    Wk = np.asarray(Wk, np.float32)
    bk = np.asarray(bk, np.float32)
    Wv = np.asarray(Wv, np.float32)
    bv = np.asarray(bv, np.float32)
    g = float(np.asarray(gamma, np.float32)[0])

    C = 256
    e = (g * np.linalg.solve(np.eye(C, dtype=np.float64) + g * Wv.astype(np.float64),
                             bv.astype(np.float64))).astype(np.float32)
    xb = (x + e[None, :, None, None]).astype(ml_dtypes.bfloat16)  # (16,256,96,96)
    # w-major: free = w*H + h
    xw = np.ascontiguousarray(xb.transpose(0, 1, 3, 2)).reshape(16, 2, P, HW)
    # per-column transposed: [b, h(g), w*256+c]
    xt = np.ascontiguousarray(xb.transpose(0, 2, 3, 1)).reshape(16, H, W * 256)

    cblob = np.zeros((P, 866), np.float32)
    cblob[:, 0:128] = np.stack([Wq[:, :128].T, Wq[:, 128:].T], axis=1).reshape(P, 128)
    cblob[:, 128:256] = np.stack([Wk[:, :128].T, Wk[:, 128:].T], axis=1).reshape(P, 128)
    WvT = Wv.T  # [c', c]
    quarters = [WvT[ci * 128:(ci + 1) * 128, mi * 128:(mi + 1) * 128]
                for ci in range(2) for mi in range(2)]  # order ci*2+mi
    cblob[:, 256:768] = np.stack(quarters, axis=1).reshape(P, 512)
    cblob[0:H, 768:864] = 1.0 / g
    cblob = cblob.astype(ml_dtypes.bfloat16)
    bblob = np.zeros((64, 2), np.float32)
    bblob[:, 0] = bq - Wq @ e
    bblob[:, 1] = bk - Wk @ e
    return xw, xt, cblob, bblob


def kernel(x, Wq, bq, Wk, bk, Wv, bv, gamma):
    xw, xt, cblob, bblob = _prep(x, Wq, bq, Wk, bk, Wv, bv, gamma)
    nc = _get_nc()
    in_maps = []
    for core in range(8):
        in_maps.append({
            "xw": xw[core * B_LOC:(core + 1) * B_LOC],
            "xt": xt[core * B_LOC:(core + 1) * B_LOC],
            "cblob": cblob, "bblob": bblob,
        })
    res = run_bass_kernel_spmd(nc, in_maps, core_ids=list(range(8)))
    outs = [np.asarray(r["out"], np.float32).reshape(B_LOC, 256, W, H)
            for r in res.results]
    full = np.concatenate(outs, axis=0)          # (16, 256, w, h)
    return np.ascontiguousarray(full.transpose(0, 1, 3, 2))


def prepared_in_maps(inputs):
    """test-harness helper: the per-core in_maps for a full input dict."""
    xw, xt, cblob, bblob = _prep(
        inputs["x"], inputs["Wq"], inputs["bq"], inputs["Wk"], inputs["bk"],
        inputs["Wv"], inputs["bv"], inputs["gamma"])
    return [
        {"xw": xw[c * B_LOC:(c + 1) * B_LOC],
         "xt": xt[c * B_LOC:(c + 1) * B_LOC],
         "cblob": cblob, "bblob": bblob}
        for c in range(8)
    ]


def postprocess(res):
    outs = [np.asarray(r["out"], np.float32).reshape(B_LOC, 256, W, H)
            for r in res.results]
    full = np.concatenate(outs, axis=0)
    return np.ascontiguousarray(full.transpose(0, 1, 3, 2))
